# revision 1
# baseline (speedup 1.0000x reference)
"""Trainium2 Bass kernel for OldNeighborhoodEncoder (segment_reduce).

Math (reference):
    fc1    = relu(X @ W1.T + b1)            # [N, 64], X = [N, 3]
    pooled = segment_max(fc1, cluster, S)   # [S, 64], cluster = arange(N)//32
    h      = relu(pooled @ W1g.T + b1g)     # [S, 64]
    out    = relu(h @ W2g.T + b2g)          # [S, 128]

Hardcoded sizes: N=1048576, S=32768 (32 pts/cluster), FEATURE=64, FG0=64,
FG1=128, 8 cores. Data-parallel over points: core d handles points
[d*131072, (d+1)*131072) == clusters [d*4096, (d+1)*4096); no collectives.

Device layout (per core):
  xt [6, 65536]: col c = 512*g + o (g in 0..127, o in 0..511); rows 0-2 =
    xyz of point 1024*g + o, rows 3-5 = xyz of point 1024*g + 512 + o.
  wpack [6,128] = blockdiag(W1.T, W1.T): one matmul column-block computes
    fc1 (pre-bias) for TWO 512-point chunks at once -> full 128-partition
    PE output. Bias+relu are deferred past the max (monotone).
  psum [128,4,16,32]: bank b holds g = 4i+b; view [.., q, t] with o=32q+t,
    so a single DVE reduce over t pools 4*16 = 64 cluster-halves.
  pooled [128, 32, 4, 16]: pooled[64a+f, i, b, q] = max_z of cluster
    128i + 32b + 16a + q, feature f.
  Tail: relu(+b1) -> blockdiag(W1g.T) matmul -> relu(+b1g) ->
    W2g.T matmul (K=64, separately for a=0 from partitions 0:64 and a=1
    from 64:128) -> relu(+b2g) -> outA/outB [128, 2048].

v1.5 perf structure: the main loop is DVE-reduce-bound (Pool/GPSIMD has no
legal max op on this target, so DVE does all 32 chunk reductions); weight
DMAs go on the Scalar queue (HWDGE; gpsimd SWDGE blocked the first matmul
~7us); relu(+b1) of pooled happens in slices during the main loop on ACT;
the tail MLP is pipelined in 512-col sub-slices with relu work split
between ACT and DVE, and output DMAs are split in halves on two queues.
"""

import sys
import numpy as np

if "/opt/trn_rl_repo" not in sys.path:
    sys.path.insert(0, "/opt/trn_rl_repo")

N = 1048576
S = 32768
PTS_PER_CLUSTER = 32
FEATURE = 64
FG0 = 64
FG1 = 128
NCORES = 8
NPC = N // NCORES          # 131072 points per core
SPC = S // NCORES          # 4096 clusters per core
G = NPC // 1024            # 128 column-groups of 512
NCHUNK = 32                # psum chunks per core (each = 4 groups)

USE_F32R = True

_PROGRAM = None  # (nc, input_names) cache


def _build_program():
    from concourse import bacc, bass, tile

    mybir = bass.mybir
    f32 = mybir.dt.float32
    # float32r: fp32 bits, full-rate (1 cycle/row) PE mode. The BIR verifier
    # requires every producer of an f32r matmul operand to emit f32r, so the
    # DRAM tensors / SBUF tiles on matmul paths are declared f32r outright.
    fmm = mybir.dt.float32r if USE_F32R else f32
    AX = mybir.AxisListType

    nc = bacc.Bacc("TRN2", target_bir_lowering=False, debug=False)

    xt = nc.dram_tensor("xt", [6, G * 512], fmm, kind="ExternalInput").ap()
    wpack = nc.dram_tensor("wpack", [6, 128], fmm, kind="ExternalInput").ap()
    b1d = nc.dram_tensor("b1d", [128, 1], f32, kind="ExternalInput").ap()
    w1gbd = nc.dram_tensor("w1gbd", [128, 128], fmm, kind="ExternalInput").ap()
    b1gd = nc.dram_tensor("b1gd", [128, 1], f32, kind="ExternalInput").ap()
    w2gt = nc.dram_tensor("w2gt", [128, 128], fmm, kind="ExternalInput").ap()
    b2g = nc.dram_tensor("b2g", [128, 1], f32, kind="ExternalInput").ap()
    outA = nc.dram_tensor("outA", [128, 2048], f32, kind="ExternalOutput").ap()
    outB = nc.dram_tensor("outB", [128, 2048], f32, kind="ExternalOutput").ap()

    with tile.TileContext(nc) as tc:
        with (
            tc.tile_pool(name="w", bufs=1) as wp,
            tc.tile_pool(name="x", bufs=3) as xp,
            tc.tile_pool(name="acc", bufs=1) as accp,
            tc.tile_pool(name="ps", bufs=2, space=bass.MemorySpace.PSUM) as pp,
        ):
            wpack_t = wp.tile([6, 128], fmm, tag="wpack")
            b1d_t = wp.tile([128, 1], f32, tag="b1d")
            w1gbd_t = wp.tile([128, 128], fmm, tag="w1gbd")
            b1gd_t = wp.tile([128, 1], f32, tag="b1gd")
            w2gt_t = wp.tile([128, 128], fmm, tag="w2gt")
            b2g_t = wp.tile([128, 1], f32, tag="b2g")
            # weight DMAs on the Scalar queue (HWDGE); wpack first — it
            # gates the first matmul.
            for t, d in (
                (wpack_t, wpack),
                (b1d_t, b1d),
                (w1gbd_t, w1gbd),
                (b1gd_t, b1gd),
                (w2gt_t, w2gt),
                (b2g_t, b2g),
            ):
                nc.scalar.dma_start(t[:], d[:])

            pooled = accp.tile([128, NCHUNK, 4, 16], f32, tag="pooled")
            pooledR = accp.tile([128, 2048], fmm, tag="pooledR")

            # main loop: fc1 matmuls + segment-max pooling
            for k in range(8):  # 8 DMA chunks of [6, 8192]
                xt_t = xp.tile([6, 8192], fmm, tag="xt")
                if k == 0:
                    # split so the first matmul's columns land early
                    nc.sync.dma_start(xt_t[:, 0:2048], xt[:, 0:2048])
                    nc.sync.dma_start(xt_t[:, 2048:8192], xt[:, 2048:8192])
                else:
                    nc.sync.dma_start(xt_t[:], xt[:, k * 8192 : (k + 1) * 8192])
                for m in range(4):
                    i = 4 * k + m
                    ps = pp.tile([128, 4, 16, 32], f32, tag="ps")
                    for b in range(4):
                        c0 = (4 * m + b) * 512
                        nc.tensor.matmul(
                            ps[:, b],
                            wpack_t[:],
                            xt_t[:, c0 : c0 + 512],
                        )
                    # all reduces on DVE: it is the only engine with a
                    # free-axis max on this target (Pool/GPSIMD rejects
                    # TensorTensor/TensorReduce/InstPool at the ISA level)
                    nc.vector.reduce_max(pooled[:, i], ps[:], axis=AX.X)
                    if i % 8 == 2 and i > 8:
                        # relu(+b1) an eighth of pooled once its chunks are
                        # done; deferred two chunks so the ACT-queue wait
                        # can't stall the next eviction.
                        s = i // 8 - 1
                        nc.scalar.activation(
                            pooledR[:, s * 512 : (s + 1) * 512],
                            pooled[:, s * 8 : (s + 1) * 8],
                            mybir.ActivationFunctionType.Relu,
                            bias=b1d_t[:],
                        )

            # last eighth of pooledR
            nc.scalar.activation(
                pooledR[:, 1536:2048],
                pooled[:, 24:32],
                mybir.ActivationFunctionType.Relu,
                bias=b1d_t[:],
            )

            # tail MLP, pipelined in 512-col sub-slices
            hps = pp.tile([128, 4, 16, 32], f32, tag="ps")
            hR = accp.tile([128, 2048], fmm, tag="hR")
            for j in range(4):
                nc.tensor.matmul(
                    hps[:, j],
                    w1gbd_t[:],
                    pooledR[:, j * 512 : (j + 1) * 512],
                )
                nc.scalar.activation(
                    hR[:, j * 512 : (j + 1) * 512],
                    hps[:, j],
                    mybir.ActivationFunctionType.Relu,
                    bias=b1gd_t[:],
                )

            opsA = pp.tile([128, 4, 16, 32], f32, tag="ps")
            opsB = pp.tile([128, 4, 16, 32], f32, tag="ps")
            o2A = accp.tile([128, 2048], f32, tag="o2A")
            o2B = accp.tile([128, 2048], f32, tag="o2B")
            add = mybir.AluOpType.add
            vmax = mybir.AluOpType.max
            for j in range(4):
                nc.tensor.matmul(
                    opsA[:, j],
                    w2gt_t[0:64, :],
                    hR[0:64, j * 512 : (j + 1) * 512],
                )
                nc.tensor.matmul(
                    opsB[:, j],
                    w2gt_t[64:128, :],
                    hR[64:128, j * 512 : (j + 1) * 512],
                )
                # relu(+b2g): o2A + first half of o2B on DVE, rest on ACT
                nc.vector.tensor_scalar(
                    o2A[:, j * 512 : (j + 1) * 512],
                    opsA[:, j], b2g_t[:], 0.0, op0=add, op1=vmax,
                )
                if j < 2:
                    nc.vector.tensor_scalar(
                        o2B[:, j * 512 : (j + 1) * 512],
                        opsB[:, j], b2g_t[:], 0.0, op0=add, op1=vmax,
                    )
                else:
                    nc.scalar.activation(
                        o2B[:, j * 512 : (j + 1) * 512],
                        opsB[:, j],
                        mybir.ActivationFunctionType.Relu,
                        bias=b2g_t[:],
                    )
                if j == 1:
                    nc.sync.dma_start(outA[:, 0:1024], o2A[:, 0:1024])
                if j == 2:
                    # after the j==2 ACT so the issue's wait on DVE's
                    # o2B slices can't stall ACT compute
                    nc.scalar.dma_start(outB[:, 0:1024], o2B[:, 0:1024])
            nc.sync.dma_start(outA[:, 1024:2048], o2A[:, 1024:2048])
            nc.scalar.dma_start(outB[:, 1024:2048], o2B[:, 1024:2048])

    nc.compile()
    return nc


def _get_program():
    global _PROGRAM
    if _PROGRAM is None:
        _PROGRAM = _build_program()
    return _PROGRAM


def _host_pack(relative_points, W1, b1, W1g, b1g, W2g, b2g):
    X = np.ascontiguousarray(relative_points, dtype=np.float32)
    W1 = np.asarray(W1, np.float32)
    b1 = np.asarray(b1, np.float32)
    W1g = np.asarray(W1g, np.float32)
    b1g = np.asarray(b1g, np.float32)
    W2g = np.asarray(W2g, np.float32)
    b2g = np.asarray(b2g, np.float32)

    wpack = np.zeros((6, 128), np.float32)
    wpack[0:3, 0:64] = W1.T
    wpack[3:6, 64:128] = W1.T
    b1d = np.concatenate([b1, b1]).reshape(128, 1)
    w1gbd = np.zeros((128, 128), np.float32)
    w1gbd[0:64, 0:64] = W1g.T
    w1gbd[64:128, 64:128] = W1g.T
    b1gd = np.concatenate([b1g, b1g]).reshape(128, 1)
    w2gt = np.ascontiguousarray(np.vstack([W2g.T, W2g.T]))  # [128, 128]
    b2gc = np.ascontiguousarray(b2g.reshape(128, 1))

    in_maps = []
    for d in range(NCORES):
        Xc = X[d * NPC : (d + 1) * NPC]
        xt6 = np.ascontiguousarray(
            Xc.reshape(G, 2, 512, 3).transpose(1, 3, 0, 2).reshape(6, G * 512)
        )
        in_maps.append(
            {
                "xt": xt6,
                "wpack": wpack,
                "b1d": b1d,
                "w1gbd": w1gbd,
                "b1gd": b1gd,
                "w2gt": w2gt,
                "b2g": b2gc,
            }
        )
    return in_maps


def _host_unpack(results):
    out = np.empty((S, FG1), np.float32)
    for d in range(NCORES):
        oA = results[d]["outA"].reshape(128, NCHUNK, 4, 16)
        oB = results[d]["outB"].reshape(128, NCHUNK, 4, 16)
        blk = out[d * SPC : (d + 1) * SPC].reshape(NCHUNK, 4, 2, 16, 128)
        blk[:, :, 0] = oA.transpose(1, 2, 3, 0)
        blk[:, :, 1] = oB.transpose(1, 2, 3, 0)
    return out


def _numpy_fallback(relative_points, cluster, num_clusters,
                    W1, b1, W1g, b1g, W2g, b2g):
    X = np.asarray(relative_points, np.float32)
    fc1 = np.maximum(X @ np.asarray(W1, np.float32).T + np.asarray(b1, np.float32), 0.0)
    Sn = int(num_clusters)
    cl = np.asarray(cluster).astype(np.int64)
    pooled = np.full((Sn, fc1.shape[1]), -np.inf, np.float32)
    # sorted segment ids -> reduceat over run starts
    starts = np.flatnonzero(np.r_[True, cl[1:] != cl[:-1]])
    seg_ids = cl[starts]
    pooled[seg_ids] = np.maximum.reduceat(fc1, starts, axis=0)
    h = np.maximum(pooled @ np.asarray(W1g, np.float32).T + np.asarray(b1g, np.float32), 0.0)
    return np.maximum(h @ np.asarray(W2g, np.float32).T + np.asarray(b2g, np.float32), 0.0).astype(np.float32)


def _run_hw(in_maps, trace=False):
    from concourse.bass_utils import run_bass_kernel_spmd

    nc = _get_program()
    return run_bass_kernel_spmd(
        nc, in_maps, list(range(NCORES)), trace=trace
    )


def kernel(relative_points, cluster, num_clusters,
           W1, b1, W1g, b1g, W2g, b2g):
    cl = np.asarray(cluster)
    expected_cl = np.arange(N, dtype=np.int64) // PTS_PER_CLUSTER
    if (
        relative_points.shape != (N, 3)
        or int(num_clusters) != S
        or not np.array_equal(cl, expected_cl)
    ):
        return _numpy_fallback(relative_points, cluster, num_clusters,
                               W1, b1, W1g, b1g, W2g, b2g)

    in_maps = _host_pack(relative_points, W1, b1, W1g, b1g, W2g, b2g)
    res = _run_hw(in_maps, trace=False)
    return _host_unpack(res.results)


def run_traced(inputs):
    """test.py helper: returns (output, exec_time_ns)."""
    in_maps = _host_pack(
        inputs["relative_points"], inputs["W1"], inputs["b1"],
        inputs["W1g"], inputs["b1g"], inputs["W2g"], inputs["b2g"],
    )
    res = _run_hw(in_maps, trace=True)
    return _host_unpack(res.results), res.exec_time_ns



# revision 2
# speedup vs baseline: 1.0158x; 1.0158x over previous
"""Trainium2 Bass kernel for OldNeighborhoodEncoder (segment_reduce).

Math (reference):
    fc1    = relu(X @ W1.T + b1)            # [N, 64], X = [N, 3]
    pooled = segment_max(fc1, cluster, S)   # [S, 64], cluster = arange(N)//32
    h      = relu(pooled @ W1g.T + b1g)     # [S, 64]
    out    = relu(h @ W2g.T + b2g)          # [S, 128]

Hardcoded sizes: N=1048576, S=32768 (32 pts/cluster), FEATURE=64, FG0=64,
FG1=128, 8 cores. Data-parallel over points: core d handles points
[d*131072, (d+1)*131072) == clusters [d*4096, (d+1)*4096); no collectives.

v2 design (from HW micro-benchmarks):
  * fc1 matmuls in fp16 (1 cyc/row like f32r, ~2^-11 rel precision):
    xt [6, 65536] fp16, wpack [6,128] fp16 blockdiag. 128 matmuls x 512
    free into psum chunks [128, 4(banks), 16(q), 32(t)]; group of 32
    consecutive t-columns = one cluster, partition halves = A/B cluster
    ranges (no cross-partition max anywhere).
  * pooling: PSUM is drained by ACT + DVE in parallel (tensor ops may
    read only ONE operand from PSUM; GPSIMD/InstPool are ISA-illegal):
      - A-chunks (even): ACT copies the whole chunk to SBUF bf16 (two
        1024-free ops so the copy starts after bank1), DVE does L1
        max(t0:16, t16:32) at bf16 2x.
      - D-chunks (odd): ACT copies only t=16:32 to bf16; DVE does L1 as
        tensor_max(psum-half, sbuf-half).
    L1 results land in a 4-chunk accumulator; one batched bf16 tree
    (8->4->2->1) then yields pooled[128, 4chunks, 4, 16] bf16.
    relu(+b1) is applied in one deferred pass per 8 chunks
    (bf16 -> f32r pooledR), which also keeps the tail identical to v1.
  * tail MLP kept from v1 (f32r): blockdiag W1g, split W2g A/B halves,
    relu work split ACT/DVE, outputs DMA'd in halves on two queues.
"""

import sys
import numpy as np

if "/opt/trn_rl_repo" not in sys.path:
    sys.path.insert(0, "/opt/trn_rl_repo")

N = 1048576
S = 32768
PTS_PER_CLUSTER = 32
FEATURE = 64
FG0 = 64
FG1 = 128
NCORES = 8
NPC = N // NCORES          # 131072 points per core
SPC = S // NCORES          # 4096 clusters per core
G = NPC // 1024            # 128 column-groups of 512
NCHUNK = 32                # psum chunks per core (each = 4 banks)

_PROGRAM = None


def _build_program():
    from concourse import bacc, bass, tile

    mybir = bass.mybir
    f32 = mybir.dt.float32
    f16 = mybir.dt.float16
    bf16 = mybir.dt.bfloat16
    fmm = mybir.dt.float32r  # tail matmuls stay f32r as in v1
    Relu = mybir.ActivationFunctionType.Relu
    Copy = mybir.ActivationFunctionType.Copy

    nc = bacc.Bacc("TRN2", target_bir_lowering=False, debug=False)

    xt = nc.dram_tensor("xt", [6, G * 512], f16, kind="ExternalInput").ap()
    wpack = nc.dram_tensor("wpack", [6, 128], f16, kind="ExternalInput").ap()
    b1d = nc.dram_tensor("b1d", [128, 1], f32, kind="ExternalInput").ap()
    w1gbd = nc.dram_tensor("w1gbd", [128, 128], fmm, kind="ExternalInput").ap()
    b1gd = nc.dram_tensor("b1gd", [128, 1], f32, kind="ExternalInput").ap()
    w2gt = nc.dram_tensor("w2gt", [128, 128], fmm, kind="ExternalInput").ap()
    b2g = nc.dram_tensor("b2g", [128, 1], f32, kind="ExternalInput").ap()
    outA = nc.dram_tensor("outA", [128, 2048], f32, kind="ExternalOutput").ap()
    outB = nc.dram_tensor("outB", [128, 2048], f32, kind="ExternalOutput").ap()

    with tile.TileContext(nc) as tc:
        with (
            tc.tile_pool(name="w", bufs=1) as wp,
            tc.tile_pool(name="x", bufs=3) as xp,
            tc.tile_pool(name="ab", bufs=3) as ap_,
            tc.tile_pool(name="ma", bufs=2) as mp,
            tc.tile_pool(name="tr", bufs=2) as tp,
            tc.tile_pool(name="acc", bufs=1) as accp,
            tc.tile_pool(name="ps", bufs=2, space=bass.MemorySpace.PSUM) as pp,
        ):
            wpack_t = wp.tile([6, 128], f16, tag="wpack")
            b1d_t = wp.tile([128, 1], f32, tag="b1d")
            w1gbd_t = wp.tile([128, 128], fmm, tag="w1gbd")
            b1gd_t = wp.tile([128, 1], f32, tag="b1gd")
            w2gt_t = wp.tile([128, 128], fmm, tag="w2gt")
            b2g_t = wp.tile([128, 1], f32, tag="b2g")
            for t, d in (
                (wpack_t, wpack),
                (b1d_t, b1d),
                (w1gbd_t, w1gbd),
                (b1gd_t, b1gd),
                (w2gt_t, w2gt),
                (b2g_t, b2g),
            ):
                nc.scalar.dma_start(t[:], d[:])

            pooled = accp.tile([128, NCHUNK, 4, 16], bf16, tag="pooled")
            pooledR = accp.tile([128, 2048], fmm, tag="pooledR")

            mA = None
            for k in range(8):  # 8 DMA chunks of [6, 8192]
                xt_t = xp.tile([6, 8192], f16, tag="xt")
                if k == 0:
                    nc.sync.dma_start(xt_t[:, 0:2048], xt[:, 0:2048])
                    nc.sync.dma_start(xt_t[:, 2048:8192], xt[:, 2048:8192])
                else:
                    nc.sync.dma_start(xt_t[:], xt[:, k * 8192 : (k + 1) * 8192])
                for m in range(4):
                    i = 4 * k + m
                    c = i % 4
                    if c == 0:
                        mA = mp.tile([128, 4, 4, 16, 16], bf16, tag="mA")
                    ps = pp.tile([128, 4, 16, 32], f32, tag="ps")
                    for b in range(4):
                        c0 = (4 * m + b) * 512
                        nc.tensor.matmul(
                            ps[:, b],
                            wpack_t[:],
                            xt_t[:, c0 : c0 + 512],
                        )
                    abf = ap_.tile([128, 4, 16, 32], bf16, tag="abf")
                    if i % 2 == 0:
                        # A-chunk: ACT copies all of ps (raw, bias deferred)
                        # in two halves so the first starts after bank 1.
                        nc.scalar.activation(abf[:, 0:2], ps[:, 0:2], Copy)
                        nc.scalar.activation(abf[:, 2:4], ps[:, 2:4], Copy)
                        nc.vector.tensor_max(
                            mA[:, c],
                            abf[:, :, :, 0:16],
                            abf[:, :, :, 16:32],
                        )
                    else:
                        # D-chunk: ACT copies only the t-high half; DVE
                        # merges psum t-low against it.
                        nc.scalar.activation(
                            abf[:, :, :, 16:32], ps[:, :, :, 16:32], Copy
                        )
                        nc.vector.tensor_max(
                            mA[:, c],
                            ps[:, :, :, 0:16],
                            abf[:, :, :, 16:32],
                        )
                    if c == 3:
                        g = i // 4
                        t2 = tp.tile([128, 4, 4, 16, 8], bf16, tag="t2")
                        t3 = tp.tile([128, 4, 4, 16, 4], bf16, tag="t3")
                        t4 = tp.tile([128, 4, 4, 16, 2], bf16, tag="t4")
                        nc.vector.tensor_max(
                            t2[:], mA[:, :, :, :, 0:8], mA[:, :, :, :, 8:16]
                        )
                        nc.vector.tensor_max(
                            t3[:], t2[:, :, :, :, 0:4], t2[:, :, :, :, 4:8]
                        )
                        nc.vector.tensor_max(
                            t4[:], t3[:, :, :, :, 0:2], t3[:, :, :, :, 2:4]
                        )
                        nc.vector.tensor_max(
                            pooled[:, 4 * g : 4 * g + 4],
                            t4[:, :, :, :, 0],
                            t4[:, :, :, :, 1],
                        )
                    if i % 8 == 2 and i > 8:
                        # deferred relu(+b1) of a ready eighth of pooled
                        s = i // 8 - 1
                        nc.scalar.activation(
                            pooledR[:, s * 512 : (s + 1) * 512],
                            pooled[:, s * 8 : (s + 1) * 8],
                            Relu,
                            bias=b1d_t[:],
                        )

            nc.scalar.activation(
                pooledR[:, 1536:2048],
                pooled[:, 24:32],
                Relu,
                bias=b1d_t[:],
            )

            # tail MLP (v1 structure, f32r)
            hps = pp.tile([128, 4, 16, 32], f32, tag="ps")
            hR = accp.tile([128, 2048], fmm, tag="hR")
            for j in range(4):
                nc.tensor.matmul(
                    hps[:, j],
                    w1gbd_t[:],
                    pooledR[:, j * 512 : (j + 1) * 512],
                )
                nc.scalar.activation(
                    hR[:, j * 512 : (j + 1) * 512],
                    hps[:, j],
                    Relu,
                    bias=b1gd_t[:],
                )

            opsA = pp.tile([128, 4, 16, 32], f32, tag="ps")
            opsB = pp.tile([128, 4, 16, 32], f32, tag="ps")
            o2A = accp.tile([128, 2048], f32, tag="o2A")
            o2B = accp.tile([128, 2048], f32, tag="o2B")
            add = mybir.AluOpType.add
            vmax = mybir.AluOpType.max
            for j in range(4):
                nc.tensor.matmul(
                    opsA[:, j],
                    w2gt_t[0:64, :],
                    hR[0:64, j * 512 : (j + 1) * 512],
                )
                nc.tensor.matmul(
                    opsB[:, j],
                    w2gt_t[64:128, :],
                    hR[64:128, j * 512 : (j + 1) * 512],
                )
                nc.vector.tensor_scalar(
                    o2A[:, j * 512 : (j + 1) * 512],
                    opsA[:, j], b2g_t[:], 0.0, op0=add, op1=vmax,
                )
                if j < 2:
                    nc.vector.tensor_scalar(
                        o2B[:, j * 512 : (j + 1) * 512],
                        opsB[:, j], b2g_t[:], 0.0, op0=add, op1=vmax,
                    )
                else:
                    nc.scalar.activation(
                        o2B[:, j * 512 : (j + 1) * 512],
                        opsB[:, j],
                        Relu,
                        bias=b2g_t[:],
                    )
                if j == 1:
                    nc.sync.dma_start(outA[:, 0:1024], o2A[:, 0:1024])
                if j == 2:
                    nc.scalar.dma_start(outB[:, 0:1024], o2B[:, 0:1024])
            nc.sync.dma_start(outA[:, 1024:2048], o2A[:, 1024:2048])
            nc.scalar.dma_start(outB[:, 1024:2048], o2B[:, 1024:2048])

    nc.compile()
    return nc


def _get_program():
    global _PROGRAM
    if _PROGRAM is None:
        _PROGRAM = _build_program()
    return _PROGRAM


def _host_pack(relative_points, W1, b1, W1g, b1g, W2g, b2g):
    X = np.ascontiguousarray(relative_points, dtype=np.float32)
    W1 = np.asarray(W1, np.float32)
    b1 = np.asarray(b1, np.float32)
    W1g = np.asarray(W1g, np.float32)
    b1g = np.asarray(b1g, np.float32)
    W2g = np.asarray(W2g, np.float32)
    b2g = np.asarray(b2g, np.float32)

    wpack = np.zeros((6, 128), np.float16)
    wpack[0:3, 0:64] = W1.T.astype(np.float16)
    wpack[3:6, 64:128] = W1.T.astype(np.float16)
    b1d = np.concatenate([b1, b1]).reshape(128, 1)
    w1gbd = np.zeros((128, 128), np.float32)
    w1gbd[0:64, 0:64] = W1g.T
    w1gbd[64:128, 64:128] = W1g.T
    b1gd = np.concatenate([b1g, b1g]).reshape(128, 1)
    w2gt = np.ascontiguousarray(np.vstack([W2g.T, W2g.T]))  # [128, 128]
    b2gc = np.ascontiguousarray(b2g.reshape(128, 1))

    in_maps = []
    for d in range(NCORES):
        Xc = X[d * NPC : (d + 1) * NPC]
        xt6 = np.ascontiguousarray(
            Xc.reshape(G, 2, 512, 3).transpose(1, 3, 0, 2).reshape(6, G * 512)
        ).astype(np.float16)
        in_maps.append(
            {
                "xt": xt6,
                "wpack": wpack,
                "b1d": b1d,
                "w1gbd": w1gbd,
                "b1gd": b1gd,
                "w2gt": w2gt,
                "b2g": b2gc,
            }
        )
    return in_maps


def _host_unpack(results):
    out = np.empty((S, FG1), np.float32)
    for d in range(NCORES):
        oA = results[d]["outA"].reshape(128, NCHUNK, 4, 16)
        oB = results[d]["outB"].reshape(128, NCHUNK, 4, 16)
        blk = out[d * SPC : (d + 1) * SPC].reshape(NCHUNK, 4, 2, 16, 128)
        blk[:, :, 0] = oA.transpose(1, 2, 3, 0)
        blk[:, :, 1] = oB.transpose(1, 2, 3, 0)
    return out


def _numpy_fallback(relative_points, cluster, num_clusters,
                    W1, b1, W1g, b1g, W2g, b2g):
    X = np.asarray(relative_points, np.float32)
    fc1 = np.maximum(X @ np.asarray(W1, np.float32).T + np.asarray(b1, np.float32), 0.0)
    Sn = int(num_clusters)
    cl = np.asarray(cluster).astype(np.int64)
    pooled = np.full((Sn, fc1.shape[1]), -np.inf, np.float32)
    starts = np.flatnonzero(np.r_[True, cl[1:] != cl[:-1]])
    seg_ids = cl[starts]
    pooled[seg_ids] = np.maximum.reduceat(fc1, starts, axis=0)
    h = np.maximum(pooled @ np.asarray(W1g, np.float32).T + np.asarray(b1g, np.float32), 0.0)
    return np.maximum(h @ np.asarray(W2g, np.float32).T + np.asarray(b2g, np.float32), 0.0).astype(np.float32)


def _run_hw(in_maps, trace=False):
    from concourse.bass_utils import run_bass_kernel_spmd

    nc = _get_program()
    return run_bass_kernel_spmd(
        nc, in_maps, list(range(NCORES)), trace=trace
    )


def kernel(relative_points, cluster, num_clusters,
           W1, b1, W1g, b1g, W2g, b2g):
    cl = np.asarray(cluster)
    expected_cl = np.arange(N, dtype=np.int64) // PTS_PER_CLUSTER
    if (
        relative_points.shape != (N, 3)
        or int(num_clusters) != S
        or not np.array_equal(cl, expected_cl)
    ):
        return _numpy_fallback(relative_points, cluster, num_clusters,
                               W1, b1, W1g, b1g, W2g, b2g)

    in_maps = _host_pack(relative_points, W1, b1, W1g, b1g, W2g, b2g)
    res = _run_hw(in_maps, trace=False)
    return _host_unpack(res.results)


def run_traced(inputs):
    """test.py helper: returns (output, exec_time_ns)."""
    in_maps = _host_pack(
        inputs["relative_points"], inputs["W1"], inputs["b1"],
        inputs["W1g"], inputs["b1g"], inputs["W2g"], inputs["b2g"],
    )
    res = _run_hw(in_maps, trace=True)
    return _host_unpack(res.results), res.exec_time_ns


# revision 5
# speedup vs baseline: 1.2076x; 1.1888x over previous
"""Trainium2 Bass kernel for OldNeighborhoodEncoder (segment_reduce).

Math (reference):
    fc1    = relu(X @ W1.T + b1)            # [N, 64], X = [N, 3]
    pooled = segment_max(fc1, cluster, S)   # [S, 64], cluster = arange(N)//32
    h      = relu(pooled @ W1g.T + b1g)     # [S, 64]
    out    = relu(h @ W2g.T + b2g)          # [S, 128]

Hardcoded sizes: N=1048576, S=32768 (32 pts/cluster), FEATURE=64, FG0=64,
FG1=128, 8 cores. Data-parallel over points: core d handles points
[d*131072, (d+1)*131072) == clusters [d*4096, (d+1)*4096); no collectives.

v2 design (from HW micro-benchmarks):
  * fc1 matmuls in fp16 (1 cyc/row like f32r, ~2^-11 rel precision):
    xt [6, 65536] fp16, wpack [6,128] fp16 blockdiag. 128 matmuls x 512
    free into psum chunks [128, 4(banks), 16(q), 32(t)]; group of 32
    consecutive t-columns = one cluster, partition halves = A/B cluster
    ranges (no cross-partition max anywhere).
  * pooling: PSUM is drained by ACT + DVE in parallel (tensor ops may
    read only ONE operand from PSUM; GPSIMD/InstPool are ISA-illegal):
      - A-chunks (even): ACT copies the whole chunk to SBUF bf16 (two
        1024-free ops so the copy starts after bank1), DVE does L1
        max(t0:16, t16:32) at bf16 2x.
      - D-chunks (odd): ACT copies only t=16:32 to bf16; DVE does L1 as
        tensor_max(psum-half, sbuf-half).
    L1 results land in a 4-chunk accumulator; one batched bf16 tree
    (8->4->2->1) then yields pooled[128, 4chunks, 4, 16] bf16.
    relu(+b1) is applied in one deferred pass per 8 chunks
    (bf16 -> f32r pooledR), which also keeps the tail identical to v1.
  * tail MLP kept from v1 (f32r): blockdiag W1g, split W2g A/B halves,
    relu work split ACT/DVE, outputs DMA'd in halves on two queues.
"""

import sys
import numpy as np

if "/opt/trn_rl_repo" not in sys.path:
    sys.path.insert(0, "/opt/trn_rl_repo")

N = 1048576
S = 32768
PTS_PER_CLUSTER = 32
FEATURE = 64
FG0 = 64
FG1 = 128
NCORES = 8
NPC = N // NCORES          # 131072 points per core
SPC = S // NCORES          # 4096 clusters per core
G = NPC // 1024            # 128 column-groups of 512
NCHUNK = 32                # psum chunks per core (each = 4 banks)

_PROGRAM = None


def _build_program():
    from concourse import bacc, bass, tile

    mybir = bass.mybir
    f32 = mybir.dt.float32
    f16 = mybir.dt.float16
    bf16 = mybir.dt.bfloat16
    fmm = mybir.dt.float32r  # tail matmuls stay f32r as in v1
    Relu = mybir.ActivationFunctionType.Relu
    Copy = mybir.ActivationFunctionType.Copy

    nc = bacc.Bacc("TRN2", target_bir_lowering=False, debug=False)

    xt = nc.dram_tensor("xt", [6, G * 512], f16, kind="ExternalInput").ap()
    wpack = nc.dram_tensor("wpack", [6, 128], f16, kind="ExternalInput").ap()
    b1d = nc.dram_tensor("b1d", [128, 1], f32, kind="ExternalInput").ap()
    w1gbd = nc.dram_tensor("w1gbd", [128, 128], fmm, kind="ExternalInput").ap()
    b1gd = nc.dram_tensor("b1gd", [128, 1], f32, kind="ExternalInput").ap()
    w2gt = nc.dram_tensor("w2gt", [128, 128], fmm, kind="ExternalInput").ap()
    b2g = nc.dram_tensor("b2g", [128, 1], f32, kind="ExternalInput").ap()
    outA = nc.dram_tensor("outA", [128, 2048], f32, kind="ExternalOutput").ap()
    outB = nc.dram_tensor("outB", [128, 2048], f32, kind="ExternalOutput").ap()

    with tile.TileContext(nc) as tc:
        with (
            tc.tile_pool(name="w", bufs=1) as wp,
            tc.tile_pool(name="x", bufs=3) as xp,
            tc.tile_pool(name="ab", bufs=3) as ap_,
            tc.tile_pool(name="ma", bufs=2) as mp,
            tc.tile_pool(name="tr", bufs=2) as tp,
            tc.tile_pool(name="acc", bufs=1) as accp,
            tc.tile_pool(name="ps", bufs=4, space=bass.MemorySpace.PSUM) as pp,
        ):
            wpack_t = wp.tile([6, 128], f16, tag="wpack")
            b1d_t = wp.tile([128, 1], f32, tag="b1d")
            w1gbd_t = wp.tile([128, 128], fmm, tag="w1gbd")
            b1gd_t = wp.tile([128, 1], f32, tag="b1gd")
            w2gt_t = wp.tile([128, 128], fmm, tag="w2gt")
            b2g_t = wp.tile([128, 1], f32, tag="b2g")
            for t, d in (
                (wpack_t, wpack),
                (b1d_t, b1d),
                (w1gbd_t, w1gbd),
                (b1gd_t, b1gd),
                (w2gt_t, w2gt),
                (b2g_t, b2g),
            ):
                nc.scalar.dma_start(t[:], d[:])

            pooled = accp.tile([128, NCHUNK, 4, 16], bf16, tag="pooled")
            pooledR = accp.tile([128, 2048], fmm, tag="pooledR")

            mA = None
            for k in range(8):  # 8 DMA chunks of [6, 8192]
                xt_t = xp.tile([6, 8192], f16, tag="xt")
                if k == 0:
                    nc.sync.dma_start(xt_t[:, 0:2048], xt[:, 0:2048])
                    nc.sync.dma_start(xt_t[:, 2048:8192], xt[:, 2048:8192])
                else:
                    nc.sync.dma_start(xt_t[:], xt[:, k * 8192 : (k + 1) * 8192])
                for m in range(8):  # 8 half-chunks of 2 banks each
                    hc = 8 * k + m
                    hcl = hc % 8
                    if hcl == 0:
                        mA = mp.tile([128, 8, 2, 16, 16], bf16, tag="mA")
                    ps = pp.tile([128, 2, 16, 32], f32, tag="ps")
                    for b in range(2):
                        c0 = (2 * m + b) * 512
                        nc.tensor.matmul(
                            ps[:, b],
                            wpack_t[:],
                            xt_t[:, c0 : c0 + 512],
                        )
                    abf = ap_.tile([128, 2, 16, 32], bf16, tag="abf")
                    if hc % 2 == 0:
                        # A-half-chunk: ACT copies both banks (raw);
                        # DVE folds t-halves at bf16 2x.
                        nc.scalar.activation(abf[:], ps[:], Copy)
                        nc.vector.tensor_max(
                            mA[:, hcl],
                            abf[:, :, :, 0:16],
                            abf[:, :, :, 16:32],
                        )
                    else:
                        # D-half-chunk: ACT copies the t-high half only;
                        # DVE merges psum t-low against it.
                        nc.scalar.activation(
                            abf[:, :, :, 16:32], ps[:, :, :, 16:32], Copy
                        )
                        nc.vector.tensor_max(
                            mA[:, hcl],
                            ps[:, :, :, 0:16],
                            abf[:, :, :, 16:32],
                        )
                    if hcl == 7:
                        # batched bf16 tree over 8 half-chunks = old
                        # chunks 4g..4g+3; mA index (hcl, bl) maps to
                        # chunk 4g + hcl//2, bank 2*(hcl%2)+bl.
                        g = hc // 8
                        t2 = tp.tile([128, 8, 2, 16, 8], bf16, tag="t2")
                        t3 = tp.tile([128, 8, 2, 16, 4], bf16, tag="t3")
                        t4 = tp.tile([128, 8, 2, 16, 2], bf16, tag="t4")
                        nc.vector.tensor_max(
                            t2[:], mA[:, :, :, :, 0:8], mA[:, :, :, :, 8:16]
                        )
                        nc.vector.tensor_max(
                            t3[:], t2[:, :, :, :, 0:4], t2[:, :, :, :, 4:8]
                        )
                        nc.vector.tensor_max(
                            t4[:], t3[:, :, :, :, 0:2], t3[:, :, :, :, 2:4]
                        )
                        nc.vector.tensor_max(
                            pooled[:, 4 * g : 4 * g + 4].rearrange(
                                "p i (pp bb) q -> p (i pp) bb q", pp=2, bb=2
                            ),
                            t4[:, :, :, :, 0],
                            t4[:, :, :, :, 1],
                        )
                    if hc % 16 == 4 and hc > 16:
                        # deferred relu(+b1) of a ready eighth of pooled
                        s = hc // 16 - 1
                        nc.scalar.activation(
                            pooledR[:, s * 512 : (s + 1) * 512],
                            pooled[:, s * 8 : (s + 1) * 8],
                            Relu,
                            bias=b1d_t[:],
                        )

            nc.scalar.activation(
                pooledR[:, 1536:2048],
                pooled[:, 24:32],
                Relu,
                bias=b1d_t[:],
            )

            # tail MLP (v1 structure, f32r; psum in 2-bank tiles)
            hR = accp.tile([128, 2048], fmm, tag="hR")
            hps2 = None
            for j in range(4):
                if j % 2 == 0:
                    hps2 = pp.tile([128, 2, 16, 32], f32, tag="ps")
                nc.tensor.matmul(
                    hps2[:, j % 2],
                    w1gbd_t[:],
                    pooledR[:, j * 512 : (j + 1) * 512],
                )
                nc.scalar.activation(
                    hR[:, j * 512 : (j + 1) * 512],
                    hps2[:, j % 2],
                    Relu,
                    bias=b1gd_t[:],
                )

            o2A = accp.tile([128, 2048], f32, tag="o2A")
            o2B = accp.tile([128, 2048], f32, tag="o2B")
            add = mybir.AluOpType.add
            vmax = mybir.AluOpType.max
            oA2 = oB2 = None
            for j in range(4):
                if j % 2 == 0:
                    oA2 = pp.tile([128, 2, 16, 32], f32, tag="ps")
                    oB2 = pp.tile([128, 2, 16, 32], f32, tag="ps")
                nc.tensor.matmul(
                    oA2[:, j % 2],
                    w2gt_t[0:64, :],
                    hR[0:64, j * 512 : (j + 1) * 512],
                )
                nc.tensor.matmul(
                    oB2[:, j % 2],
                    w2gt_t[64:128, :],
                    hR[64:128, j * 512 : (j + 1) * 512],
                )
                nc.vector.tensor_scalar(
                    o2A[:, j * 512 : (j + 1) * 512],
                    oA2[:, j % 2], b2g_t[:], 0.0, op0=add, op1=vmax,
                )
                if j < 2:
                    nc.vector.tensor_scalar(
                        o2B[:, j * 512 : (j + 1) * 512],
                        oB2[:, j % 2], b2g_t[:], 0.0, op0=add, op1=vmax,
                    )
                else:
                    nc.scalar.activation(
                        o2B[:, j * 512 : (j + 1) * 512],
                        oB2[:, j % 2],
                        Relu,
                        bias=b2g_t[:],
                    )
                if j == 1:
                    nc.sync.dma_start(outA[:, 0:1024], o2A[:, 0:1024])
                if j == 2:
                    nc.scalar.dma_start(outB[:, 0:1024], o2B[:, 0:1024])
            nc.sync.dma_start(outA[:, 1024:2048], o2A[:, 1024:2048])
            nc.scalar.dma_start(outB[:, 1024:2048], o2B[:, 1024:2048])

    nc.compile()
    return nc


def _get_program():
    global _PROGRAM
    if _PROGRAM is None:
        _PROGRAM = _build_program()
    return _PROGRAM


def _host_pack(relative_points, W1, b1, W1g, b1g, W2g, b2g):
    X = np.ascontiguousarray(relative_points, dtype=np.float32)
    W1 = np.asarray(W1, np.float32)
    b1 = np.asarray(b1, np.float32)
    W1g = np.asarray(W1g, np.float32)
    b1g = np.asarray(b1g, np.float32)
    W2g = np.asarray(W2g, np.float32)
    b2g = np.asarray(b2g, np.float32)

    wpack = np.zeros((6, 128), np.float16)
    wpack[0:3, 0:64] = W1.T.astype(np.float16)
    wpack[3:6, 64:128] = W1.T.astype(np.float16)
    b1d = np.concatenate([b1, b1]).reshape(128, 1)
    w1gbd = np.zeros((128, 128), np.float32)
    w1gbd[0:64, 0:64] = W1g.T
    w1gbd[64:128, 64:128] = W1g.T
    b1gd = np.concatenate([b1g, b1g]).reshape(128, 1)
    w2gt = np.ascontiguousarray(np.vstack([W2g.T, W2g.T]))  # [128, 128]
    b2gc = np.ascontiguousarray(b2g.reshape(128, 1))

    in_maps = []
    for d in range(NCORES):
        Xc = X[d * NPC : (d + 1) * NPC]
        xt6 = np.ascontiguousarray(
            Xc.reshape(G, 2, 512, 3).transpose(1, 3, 0, 2).reshape(6, G * 512)
        ).astype(np.float16)
        in_maps.append(
            {
                "xt": xt6,
                "wpack": wpack,
                "b1d": b1d,
                "w1gbd": w1gbd,
                "b1gd": b1gd,
                "w2gt": w2gt,
                "b2g": b2gc,
            }
        )
    return in_maps


def _host_unpack(results):
    out = np.empty((S, FG1), np.float32)
    for d in range(NCORES):
        oA = results[d]["outA"].reshape(128, NCHUNK, 4, 16)
        oB = results[d]["outB"].reshape(128, NCHUNK, 4, 16)
        blk = out[d * SPC : (d + 1) * SPC].reshape(NCHUNK, 4, 2, 16, 128)
        blk[:, :, 0] = oA.transpose(1, 2, 3, 0)
        blk[:, :, 1] = oB.transpose(1, 2, 3, 0)
    return out


def _numpy_fallback(relative_points, cluster, num_clusters,
                    W1, b1, W1g, b1g, W2g, b2g):
    X = np.asarray(relative_points, np.float32)
    fc1 = np.maximum(X @ np.asarray(W1, np.float32).T + np.asarray(b1, np.float32), 0.0)
    Sn = int(num_clusters)
    cl = np.asarray(cluster).astype(np.int64)
    pooled = np.full((Sn, fc1.shape[1]), -np.inf, np.float32)
    starts = np.flatnonzero(np.r_[True, cl[1:] != cl[:-1]])
    seg_ids = cl[starts]
    pooled[seg_ids] = np.maximum.reduceat(fc1, starts, axis=0)
    h = np.maximum(pooled @ np.asarray(W1g, np.float32).T + np.asarray(b1g, np.float32), 0.0)
    return np.maximum(h @ np.asarray(W2g, np.float32).T + np.asarray(b2g, np.float32), 0.0).astype(np.float32)


def _run_hw(in_maps, trace=False):
    from concourse.bass_utils import run_bass_kernel_spmd

    nc = _get_program()
    return run_bass_kernel_spmd(
        nc, in_maps, list(range(NCORES)), trace=trace
    )


def kernel(relative_points, cluster, num_clusters,
           W1, b1, W1g, b1g, W2g, b2g):
    cl = np.asarray(cluster)
    expected_cl = np.arange(N, dtype=np.int64) // PTS_PER_CLUSTER
    if (
        relative_points.shape != (N, 3)
        or int(num_clusters) != S
        or not np.array_equal(cl, expected_cl)
    ):
        return _numpy_fallback(relative_points, cluster, num_clusters,
                               W1, b1, W1g, b1g, W2g, b2g)

    in_maps = _host_pack(relative_points, W1, b1, W1g, b1g, W2g, b2g)
    res = _run_hw(in_maps, trace=False)
    return _host_unpack(res.results)


def run_traced(inputs):
    """test.py helper: returns (output, exec_time_ns)."""
    in_maps = _host_pack(
        inputs["relative_points"], inputs["W1"], inputs["b1"],
        inputs["W1g"], inputs["b1g"], inputs["W2g"], inputs["b2g"],
    )
    res = _run_hw(in_maps, trace=True)
    return _host_unpack(res.results), res.exec_time_ns
